# revision 8
# baseline (speedup 1.0000x reference)
"""Trainium2 Bass kernel for AttnPowerP2 (power-softmax p=2 attention).

reference:
    scores = (q @ k^T) / 8 + (1-mask)*-1e4          [B,H,T,T]
    num    = scores^2
    den    = num.sum(-1) + 1e-6
    p      = num / den
    out    = p @ v                                   [B,H,T,D]
    returns (out, p)

Sharding: batch (B=8) across the 8 NeuronCores; each core computes its
batch's 12 heads locally, no cross-core comms.

On-chip layout ("layout B"): scores are computed transposed, [k, q], so
that the k (contraction) index of the second matmul sits on the SBUF
partition dimension.  The probability matrix is written to HBM transposed
([h, k, q]) and the host unshard step returns a transposed *view* — no
data movement.  The per-q row-sum `den` falls out of the PV matmul for
free via a column of ones prepended to V.
"""

import os
import sys

os.environ.setdefault("BASS_NEVER_TRACE", "1")

try:
    import concourse  # noqa: F401
except ImportError:  # pragma: no cover
    for _p in ("/opt/trn_rl_repo", "/root/.axon_site/_ro/trn_rl_repo"):
        if _p not in sys.path:
            sys.path.insert(0, _p)

import numpy as np
from concourse import bacc, mybir
from concourse import tile
from concourse.bass_utils import run_bass_kernel_spmd

F32 = mybir.dt.float32
F32R = mybir.dt.float32r
I32 = mybir.dt.int32
AF = mybir.ActivationFunctionType

B, H, T, D = 8, 12, 1024, 64
NCORES = 8
NBLK = T // 128          # 8 k-blocks of 128
NCHUNK = T // 512        # 2 q-chunks of 512
SCALE = 1.0 / 8.0        # 1/sqrt(64)
EPS = 1e-06


def build_program(loop_n: int = 1):
    """Emit the per-core Bass program. loop_n>1 wraps the whole body in a
    hardware For_i loop (benchmarking variant)."""
    nc = bacc.Bacc("TRN2", target_bir_lowering=False, debug=False,
                   num_devices=NCORES)

    q_d = nc.dram_tensor("q", [H, T, D], F32, kind="ExternalInput")
    k_d = nc.dram_tensor("k", [H, T, D], F32, kind="ExternalInput")
    v_d = nc.dram_tensor("v", [H, T, D + 1], F32, kind="ExternalInput")
    m_d = nc.dram_tensor("mask", [T], I32, kind="ExternalInput")
    id_d = nc.dram_tensor("ident", [128, 128], F32, kind="ExternalInput")
    pt_d = nc.dram_tensor("p_t", [H, T, T], F32, kind="ExternalOutput")
    ot_d = nc.dram_tensor("out_t", [H, D, T], F32, kind="ExternalOutput")

    with tile.TileContext(nc) as tc:
        with (
            tc.tile_pool(name="const", bufs=1) as constp,
            tc.tile_pool(name="stage", bufs=2) as stagep,
            tc.tile_pool(name="qkt", bufs=2) as qktp,
            tc.tile_pool(name="vt", bufs=2) as vtp,
            tc.tile_pool(name="num", bufs=2) as nump,
            tc.tile_pool(name="small", bufs=2) as smallp,
            tc.tile_pool(name="pstr", bufs=2, space="PSUM") as pstr,
            tc.tile_pool(name="pssc", bufs=4, space="PSUM") as pssc,
            tc.tile_pool(name="psout", bufs=2, space="PSUM") as psout,
        ):
            ident = constp.tile([128, 128], F32, tag="ident")
            nc.sync.dma_start(ident[:], id_d[:])

            m_i = constp.tile([128, NBLK], I32, tag="mi")
            nc.sync.dma_start(m_i[:], m_d[:].rearrange("(b p) -> p b", p=128))
            m_f = constp.tile([128, NBLK], F32, tag="mf")
            nc.vector.tensor_copy(m_f[:], m_i[:])
            # (1-mask)*-1e4 == mask*1e4 - 1e4
            m_add = constp.tile([128, NBLK], F32, tag="madd")
            nc.vector.tensor_scalar(
                m_add[:], m_f[:], 10000.0, -10000.0,
                mybir.AluOpType.mult, mybir.AluOpType.add,
            )

            def one_head(h):
                # ---- load q,k natural layout, PE-transpose to [d, t] ----
                stage = stagep.tile([128, 1024], F32, tag="stage")
                sv = stage.rearrange("p (b x) -> p b x", x=64)
                nc.sync.dma_start(
                    sv[:, 0:8, :], q_d[h].rearrange("(b p) d -> p b d", p=128))
                nc.sync.dma_start(
                    sv[:, 8:16, :], k_d[h].rearrange("(b p) d -> p b d", p=128))

                # both q^T and k^T live on partitions 0-63 (matmul operands
                # must share base partition)
                q_t = qktp.tile([64, 1024], F32R, tag="qt")
                k_t = qktp.tile([64, 1024], F32R, tag="kt")
                for half, dst in ((0, q_t), (1, k_t)):
                    for j in range(2):
                        tr = pstr.tile([64, 512], F32, tag="tr",
                                       name=f"tr{half}{j}")
                        for i in range(4):
                            b = j * 4 + i
                            nc.tensor.transpose(
                                tr[:, i * 128:(i + 1) * 128],
                                stage[:, (half * 8 + b) * 64:
                                      (half * 8 + b + 1) * 64],
                                ident[:])
                        nc.scalar.activation(
                            dst[:, j * 512:(j + 1) * 512], tr[:], AF.Copy)

                # ---- V' = [v | 1] per k-block, fp32r (ones appended on host)
                v_t = vtp.tile([128, NBLK * 65], F32R, tag="vt")
                nc.gpsimd.dma_start(
                    v_t.rearrange("p (b x) -> p b x", x=65),
                    v_d[h].rearrange("(b p) d -> p b d", p=128))

                num_t = nump.tile([128, NBLK * 1024], F32R, tag="num")
                bcast = smallp.tile([128, 1024], F32, tag="bc")
                den = smallp.tile([65, 1024], F32, tag="den")
                den0 = smallp.tile([1, 1024], F32, tag="den0")
                rec = smallp.tile([1, 1024], F32, tag="rec")
                out_sb = smallp.tile([64, 1024], F32, tag="osb")

                for c in range(NCHUNK):
                    for b in range(NBLK):
                        ps = pssc.tile([128, 512], F32, tag="sc")
                        # scores^T[k_blk, q_chunk] = k_t^T @ q_t
                        nc.tensor.matmul(
                            ps[:],
                            k_t[:, b * 128:(b + 1) * 128],
                            q_t[:, c * 512:(c + 1) * 512],
                            start=True, stop=True)
                        # num = (scores/8 + m)^2 ; mask bias is per-partition
                        nc.scalar.activation(
                            num_t[:, b * 1024 + c * 512: b * 1024 + (c + 1) * 512],
                            ps[:], AF.Square,
                            bias=m_add[:, b:b + 1], scale=SCALE)
                    po = psout.tile([65, 512], F32, tag="po")
                    for b in range(NBLK):
                        # out'^T[0:64]=v^T @ num^T, row 64 = den (ones col)
                        nc.tensor.matmul(
                            po[:],
                            v_t[:, b * 65:(b + 1) * 65],
                            num_t[:, b * 1024 + c * 512: b * 1024 + (c + 1) * 512],
                            start=(b == 0), stop=(b == NBLK - 1))
                    cs = slice(c * 512, (c + 1) * 512)
                    nc.scalar.activation(den[64:65, cs], po[64:65, :], AF.Copy,
                                         bias=EPS)
                    # engines cannot shift partitions; DMA den row 64 -> row 0
                    nc.sync.dma_start(den0[0:1, cs], den[64:65, cs])
                    nc.vector.reciprocal(rec[0:1, cs], den0[0:1, cs])
                    nc.gpsimd.partition_broadcast(bcast[:, cs], rec[0:1, cs])
                    nc.vector.tensor_mul(out_sb[:, cs], po[0:64, :], bcast[0:64, cs])

                # p^T = num^T * (1/den) broadcast along k
                p_sb = nump.tile([128, NBLK * 1024], F32, tag="psb")
                for b in range(NBLK):
                    bs = slice(b * 1024, (b + 1) * 1024)
                    nc.vector.tensor_mul(
                        p_sb[:, bs], num_t[:, bs].bitcast(F32),
                        bcast[:, 0:1024])

                nc.sync.dma_start(
                    pt_d[h].rearrange("(b p) q -> p b q", p=128),
                    p_sb[:].rearrange("p (b q) -> p b q", q=1024))
                nc.sync.dma_start(ot_d[h], out_sb[:])

            if loop_n == 1:
                for h in range(H):
                    one_head(h)
            else:
                with tc.For_i(0, loop_n):
                    for h in range(H):
                        one_head(h)

    nc.compile()
    return nc


_CACHE = {}


def _get_program(loop_n: int = 1):
    if loop_n not in _CACHE:
        _CACHE[loop_n] = build_program(loop_n)
    return _CACHE[loop_n]


def run_cores(q, k, v, attention_mask, loop_n: int = 1):
    """Run the SPMD program; returns (out, p) with p as a transposed view."""
    nc = _get_program(loop_n)
    ident = np.eye(128, dtype=np.float32)
    v_aug = np.concatenate(
        [v, np.ones((NCORES, H, T, 1), dtype=np.float32)], axis=-1)
    v_aug = np.ascontiguousarray(v_aug)
    in_maps = [
        {
            "q": np.ascontiguousarray(q[c]),
            "k": np.ascontiguousarray(k[c]),
            "v": v_aug[c],
            "mask": np.ascontiguousarray(attention_mask[c]),
            "ident": ident,
        }
        for c in range(NCORES)
    ]
    res = run_bass_kernel_spmd(nc, in_maps, list(range(NCORES)))
    p_t = np.stack([res.results[c]["p_t"] for c in range(NCORES)])
    out_t = np.stack([res.results[c]["out_t"] for c in range(NCORES)])
    # [B,H,T,T] from [B,H,K,Q]; transposed views, no copy
    p = p_t.transpose(0, 1, 3, 2)
    out = out_t.transpose(0, 1, 3, 2)
    return out, p


def kernel(q, k, v, attention_mask):
    q = np.asarray(q, dtype=np.float32)
    k = np.asarray(k, dtype=np.float32)
    v = np.asarray(v, dtype=np.float32)
    attention_mask = np.asarray(attention_mask, dtype=np.int32)
    return run_cores(q, k, v, attention_mask, loop_n=1)
